# revision 42
# baseline (speedup 1.0000x reference)
"""CCGQA attention on 8 Trainium2 NeuronCores (Bass/Tile, SPMD).

Decomposition (uniform SPMD programs; per-core differences are data only):
  Launch 1 - core (b, g), g in 0..3: preprocessing for two folded 256-row
    query chunks {g, 7-g} of batch b (plus 4-row conv halos):
    fused QKV+head-mean GEMM, grouped conv1 + dense conv2 (shifted matmuls),
    qk-mean coupling, l2-norm (partition reduction via matmul with a
    block-ones matrix), RoPE (pair swap via permutation matmul; softmax
    1/sqrt(d) folded into q cos/sin tables, key_temp folded into k tables),
    and the V projection in row-major layout.
  Host: reassemble full K^T, Q^T, V per batch, apply the v_prev shift,
    append a ones column to V (so the attn@V matmul chain also produces the
    softmax denominator; qk-norm bounds scores to [-1,1] so exp needs no
    max subtraction).
  Launch 2 - core (b, kvh): causal attention for 4 q heads / 1 kv head over
    all 2048 rows + row-split w_o partials (bf16), summed on host.
"""
import sys
import os

sys.path.insert(0, '/opt/trn_rl_repo')

import numpy as np
import ml_dtypes

BF = ml_dtypes.bfloat16
F16 = np.float16
F32 = np.float32

B, S, DIM = 2, 2048, 4096
NH, NKV, HD = 16, 4, 64
LAT, KVD, KSZ = 1024, 256, 3
CH = 256            # fold chunk rows
HALO = 4
CW = CH + HALO      # 260 cols per chunk (with halo)
NCORES = 8

_PROGS = {}


def _dt():
    from concourse import mybir
    return mybir.dt


# ---------------------------------------------------------------------------
# program builders
# ---------------------------------------------------------------------------

def _build_launch1(with_qc2b, with_kc2b, phases=('gemm1','v','conv2','tail')):
    from contextlib import ExitStack
    import concourse.tile as tile
    from concourse import bacc, mybir

    dt = mybir.dt
    nc = bacc.Bacc('TRN2', target_bir_lowering=False, debug=False,
                   num_devices=NCORES)

    def din(name, shape, dtype=dt.bfloat16):
        return nc.dram_tensor(name, shape, dtype, kind='ExternalInput').ap()

    def dout(name, shape, dtype=dt.bfloat16):
        return nc.dram_tensor(name, shape, dtype, kind='ExternalOutput').ap()

    xt = din('xt', [128, 32, 2 * CW])
    wqk = din('wqk', [10, 128, 32, 128])
    m128 = din('m128', [128, 128])
    wv = din('wv', [128, 32, 256])
    qc1 = din('qc1', [128, 3, 8, 128])
    kc1 = din('kc1', [128, 3, 2, 128])
    qc2 = din('qc2', [8, 128, 3, 8, 128])
    kc2 = din('kc2', [2, 128, 3, 2, 128])
    csq = din('csq', [128, 2, 2, CH], dt.float16)
    csk = din('csk', [128, 2, 2, CH], dt.float16)
    e2 = din('e2', [128, 2], dt.float16)
    e2t = din('e2t', [2, 128], dt.float16)
    p128 = din('p128', [128, 128], dt.float16)
    qc1b = din('qc1b', [128, 8], dt.float32)
    kc1b = din('kc1b', [128, 2], dt.float32)
    qc2b = din('qc2b', [128, 8], dt.float32)
    kc2b = din('kc2b', [128, 2], dt.float32)

    qat = dout('qat', [LAT, 2 * CH])
    kat = dout('kat', [KVD, 2 * CH])
    vo = dout('vo', [2 * CH, KVD])

    Add = mybir.AluOpType.add
    Mult = mybir.AluOpType.mult
    Sqrt = mybir.ActivationFunctionType.Sqrt

    with tile.TileContext(nc) as tc, ExitStack() as ctx:
        cons = ctx.enter_context(tc.tile_pool(name='cons', bufs=1))
        wpool = ctx.enter_context(tc.tile_pool(name='wpool', bufs=3))
        qpre = ctx.enter_context(tc.tile_pool(name='qpre', bufs=3))
        c1buf = ctx.enter_context(tc.tile_pool(name='c1buf', bufs=1))
        tmp = ctx.enter_context(tc.tile_pool(name='tmp', bufs=4))
        small = ctx.enter_context(tc.tile_pool(name='small', bufs=4))
        outp = ctx.enter_context(tc.tile_pool(name='outp', bufs=3))
        gemm_ps = ctx.enter_context(tc.tile_pool(name='gemm_ps', bufs=2, space='PSUM'))
        mean_ps = ctx.enter_context(tc.tile_pool(name='mean_ps', bufs=1, space='PSUM'))
        conv_ps = ctx.enter_context(tc.tile_pool(name='conv_ps', bufs=2, space='PSUM'))
        nrm_ps = ctx.enter_context(tc.tile_pool(name='nrm_ps', bufs=1, space='PSUM'))

        # DMA order tuned so the first GEMM matmuls start ASAP: first weight
        # tile, then x pieces in consumption order, then the second weight
        # tile, then everything else.
        # PE warmup: dummy matmuls on a memset tile so the HAM clock-gate is
        # released (and stays released) before the first real matmul.
        wrm = cons.tile([128, 512], dt.bfloat16, name='wrm')
        nc.vector.memset(wrm, 0.0)
        wps = gemm_ps.tile([128, 512], dt.float32, tag='gps')
        for _ in range(8):
            nc.tensor.matmul(wps, wrm[:, 0:128], wrm, start=True, stop=True)

        w01 = []
        for ct in range(2):
            w = wpool.tile([128, 32, 128], dt.bfloat16, tag='wstream')
            w01.append(w)
        xt_sbs = [cons.tile([128, 8, 2 * CW], dt.bfloat16, name=f'xt_sb{xq}')
                  for xq in range(4)]
        nc.sync.dma_start(out=w01[0][:, 0:16, :], in_=wqk[0][:, 0:16, :])
        nc.sync.dma_start(out=xt_sbs[0][:, :, 0:CW], in_=xt[:, 0:8, 0:CW])
        nc.sync.dma_start(out=xt_sbs[1][:, :, 0:CW], in_=xt[:, 8:16, 0:CW])
        nc.sync.dma_start(out=w01[0][:, 16:32, :], in_=wqk[0][:, 16:32, :])
        nc.sync.dma_start(out=xt_sbs[2][:, :, 0:CW], in_=xt[:, 16:24, 0:CW])
        nc.sync.dma_start(out=xt_sbs[3][:, :, 0:CW], in_=xt[:, 24:32, 0:CW])
        nc.sync.dma_start(out=w01[1], in_=wqk[1])
        for xq in range(4):
            nc.sync.dma_start(out=xt_sbs[xq][:, :, CW:2 * CW],
                              in_=xt[:, 8 * xq:8 * (xq + 1), CW:2 * CW])
        qc1_sb = cons.tile([128, 3, 8, 128], dt.bfloat16)
        nc.sync.dma_start(out=qc1_sb, in_=qc1)
        kc1_sb = cons.tile([128, 3, 2, 128], dt.bfloat16)
        nc.sync.dma_start(out=kc1_sb, in_=kc1)
        qc1b_sb = cons.tile([128, 8], dt.float32)
        nc.sync.dma_start(out=qc1b_sb, in_=qc1b)
        kc1b_sb = cons.tile([128, 2], dt.float32)
        nc.sync.dma_start(out=kc1b_sb, in_=kc1b)
        m128_sb = cons.tile([128, 128], dt.bfloat16)
        nc.sync.dma_start(out=m128_sb, in_=m128)

        # persistent intermediates
        c1q = c1buf.tile([128, 2, 8, CH + 2], dt.bfloat16)   # conv1(q) both chunks
        c1k = c1buf.tile([128, 2, 2, CH + 2], dt.bfloat16)
        qmd = c1buf.tile([128, 2, CH], dt.float32)           # qmean/16 duplicated
        kmd = c1buf.tile([128, 2, CH], dt.float32)           # kmean/4  duplicated

        # head means accumulate in PSUM via matmuls against M128 (1/16 block
        # matrix) as each pre tile is produced; copied to SBUF afterwards.
        qm_ps = mean_ps.tile([128, 2, CH], dt.float32, tag='qmps')
        km_ps = mean_ps.tile([128, 2, CH], dt.float32, tag='kmps')

        # ---- GEMM1 (qk) + head means + grouped conv1 ----
        for ct in range(10):
            if ct < 2:
                w = w01[ct]
            else:
                w = wpool.tile([128, 32, 128], dt.bfloat16, tag='wstream')
                nc.sync.dma_start(out=w, in_=wqk[ct])
            pre = qpre.tile([128, 2, CW], dt.bfloat16, tag='pre')
            for ch in range(2):
                ps = gemm_ps.tile([128, CW], dt.float32, tag='gps')
                for kt in range(32):
                    nc.tensor.matmul(ps, w[:, kt, :],
                                     xt_sbs[kt // 8][:, kt % 8,
                                                     ch * CW:(ch + 1) * CW],
                                     start=(kt == 0), stop=(kt == 31))
                nc.vector.tensor_copy(pre[:, ch, :], ps)
                # conv1 for this tile (pair of heads)
                c1ps = conv_ps.tile([128, CH + 2], dt.float32, tag='cps')
                if ct < 8:
                    for j in range(3):
                        nc.tensor.matmul(c1ps, qc1_sb[:, j, ct, :],
                                         pre[:, ch, j:j + CH + 2],
                                         start=(j == 0), stop=(j == 2))
                    nc.vector.tensor_scalar(c1q[:, ch, ct, :], c1ps,
                                            qc1b_sb[:, ct:ct + 1], None, Add)
                else:
                    p = ct - 8
                    for j in range(3):
                        nc.tensor.matmul(c1ps, kc1_sb[:, j, p, :],
                                         pre[:, ch, j:j + CH + 2],
                                         start=(j == 0), stop=(j == 2))
                    nc.vector.tensor_scalar(c1k[:, ch, p, :], c1ps,
                                            kc1b_sb[:, p:p + 1], None, Add)
            # single open accumulation group per PSUM bank: one 2D-AP mean
            # matmul per ct covering both chunks
            if ct < 8:
                nc.tensor.matmul(qm_ps, m128_sb, pre[:, :, HALO:CW],
                                 start=(ct == 0), stop=(ct == 7))
            else:
                nc.tensor.matmul(km_ps, m128_sb, pre[:, :, HALO:CW],
                                 start=(ct == 8), stop=(ct == 9))
        nc.vector.tensor_copy(qmd, qm_ps)
        nc.vector.tensor_copy(kmd, km_ps)

        # late constants (needed from conv2-tail onward)
        csq_sb = cons.tile([128, 2, 2, CH], dt.float16)
        nc.sync.dma_start(out=csq_sb, in_=csq)
        csk_sb = cons.tile([128, 2, 2, CH], dt.float16)
        nc.sync.dma_start(out=csk_sb, in_=csk)
        e2_sb = cons.tile([128, 2], dt.float16)
        nc.sync.dma_start(out=e2_sb, in_=e2)
        e2t_sb = cons.tile([2, 128], dt.float16)
        nc.sync.dma_start(out=e2t_sb, in_=e2t)
        p128_sb = cons.tile([128, 128], dt.float16)
        nc.sync.dma_start(out=p128_sb, in_=p128)
        qc2b_sb = cons.tile([128, 8], dt.float32)
        nc.sync.dma_start(out=qc2b_sb, in_=qc2b)
        kc2b_sb = cons.tile([128, 2], dt.float32)
        nc.sync.dma_start(out=kc2b_sb, in_=kc2b)
        eps_sb = cons.tile([128, 1], dt.float32)
        nc.vector.memset(eps_sb, 1e-24)

        # ---- V projection (row-major) ----
        if 'v' in phases:
            wv_sb = cons.tile([128, 32, 256], dt.bfloat16)
            nc.sync.dma_start(out=wv_sb, in_=wv)
        for rt in range(4 if 'v' in phases else 0):
            c0 = HALO + 128 * rt if rt < 2 else CW + HALO + 128 * (rt - 2)
            ps = gemm_ps.tile([128, CW], dt.float32, tag='gps')
            for kt in range(32):
                nc.tensor.matmul(ps[:, 0:256],
                                 xt_sbs[kt // 8][:, kt % 8, c0:c0 + 128],
                                 wv_sb[:, kt, :],
                                 start=(kt == 0), stop=(kt == 31))
            vsb = outp.tile([128, 256], dt.bfloat16, tag='vout')
            nc.vector.tensor_copy(vsb, ps[:, 0:256])
            nc.sync.dma_start(out=vo[128 * rt:128 * (rt + 1), :], in_=vsb)

        # ---- conv2 + coupling + l2norm + rope ----
        def tail(oc, ch, ps2, mdup, coeff, bias_sb, with_bias, cs_sb, out_dram):
            qf = tmp.tile([128, CH], dt.float16, tag='qf')
            nc.vector.scalar_tensor_tensor(qf, mdup[:, ch, :], coeff, ps2,
                                           op0=Mult, op1=Add)
            if with_bias:
                nc.vector.tensor_scalar(qf, qf, bias_sb[:, oc:oc + 1], None, Add)
            sq = tmp.tile([128, CH], dt.float16, tag='sq')
            nc.vector.tensor_tensor(sq, qf, qf, op=Mult)
            nps = nrm_ps.tile([128, CH], dt.float32, tag='nps')
            nc.tensor.matmul(nps[0:2, :], e2_sb, sq, start=True, stop=True)
            nrm = small.tile([2, CH], dt.float32, tag='nrm')
            nc.scalar.activation(nrm, nps[0:2, :], Sqrt, bias=eps_sb[0:2, :])
            rin = small.tile([2, CH], dt.float32, tag='rin')
            nc.vector.reciprocal_approx_fast(rin, nrm)
            rin16 = small.tile([2, CH], dt.float16, tag='rin16')
            nc.scalar.copy(rin16, rin)
            bs = nrm_ps.tile([128, 2, CH], dt.float32, tag='bs')
            bps = bs[:, 0, :]
            nc.tensor.matmul(bps, e2t_sb, rin16, start=True, stop=True)
            sps = bs[:, 1, :]
            nc.tensor.matmul(sps, p128_sb, qf, start=True, stop=True)
            t1 = tmp.tile([128, CH], dt.float16, tag='t1')
            nc.vector.tensor_tensor(t1, qf, cs_sb[:, ch, 0, :], op=Mult)
            t2 = tmp.tile([128, CH], dt.float16, tag='t2')
            nc.vector.tensor_tensor(t2, sps, cs_sb[:, ch, 1, :], op=Mult)
            t3 = tmp.tile([128, CH], dt.float16, tag='t3')
            nc.vector.tensor_add(t3, t1, t2)
            qo = outp.tile([128, CH], dt.bfloat16, tag='qo')
            nc.vector.tensor_tensor(qo, t3, bps, op=Mult)
            nc.sync.dma_start(
                out=out_dram[128 * oc:128 * (oc + 1), CH * ch:CH * (ch + 1)],
                in_=qo)

        # conv2 q/k interleaved so the small k-convs' tails overlap with
        # later q-conv matmuls instead of serializing at the very end.
        seq = [('q', 0), ('q', 1), ('q', 2), ('q', 3), ('k', 0),
               ('q', 4), ('q', 5), ('k', 1), ('q', 6), ('q', 7)]
        for kind, oc in (seq if 'conv2' in phases else []):
            nit = 8 if kind == 'q' else 2
            w2 = wpool.tile([128, 3, nit, 128], dt.bfloat16, tag='wstream')
            nc.sync.dma_start(out=w2, in_=(qc2[oc] if kind == 'q' else kc2[oc]))
            c1 = c1q if kind == 'q' else c1k
            for ch in range(2):
                ps2 = conv_ps.tile([128, CH + 2], dt.float32, tag='cps')
                n = 0
                for j in range(3):
                    for it in range(nit):
                        nc.tensor.matmul(ps2[:, 0:CH], w2[:, j, it, :],
                                         c1[:, ch, it, j:j + CH],
                                         start=(n == 0), stop=(n == 3 * nit - 1))
                        n += 1
                if 'tail' in phases:
                    if kind == 'q':
                        tail(oc, ch, ps2[:, 0:CH], kmd, 2.0, qc2b_sb,
                             with_qc2b, csq_sb, qat)
                    else:
                        tail(oc, ch, ps2[:, 0:CH], qmd, 0.5, kc2b_sb,
                             with_kc2b, csk_sb, kat)
                else:
                    qo = outp.tile([128, CH], dt.bfloat16, tag='qo')
                    nc.vector.tensor_copy(qo, ps2[:, 0:CH])
                    od = qat if kind == 'q' else kat
                    nc.sync.dma_start(
                        out=od[128 * oc:128 * (oc + 1), CH * ch:CH * (ch + 1)],
                        in_=qo)

    nc.compile()
    return nc


def _build_launch2():
    from contextlib import ExitStack
    import concourse.tile as tile
    from concourse import bacc, mybir

    dt = mybir.dt
    nc = bacc.Bacc('TRN2', target_bir_lowering=False, debug=False,
                   num_devices=NCORES)

    def din(name, shape, dtype=dt.bfloat16):
        return nc.dram_tensor(name, shape, dtype, kind='ExternalInput').ap()

    qt2 = din('qt2', [128, 4, S])
    kt2 = din('kt2', [128, S])
    va2 = din('va2', [128, 16, HD + 1])
    wo2 = din('wo2', [128, 32, 2, 128])
    mc4 = din('mc4', [128, 4, 512])
    i128 = din('i128', [128, 128])
    po = nc.dram_tensor('po', [DIM, S], dt.bfloat16, kind='ExternalOutput').ap()

    Mult = mybir.AluOpType.mult
    Exp = mybir.ActivationFunctionType.Exp
    Log = mybir.ActivationFunctionType.Ln
    QC = 512                      # query chunk width

    with tile.TileContext(nc) as tc, ExitStack() as ctx:
        cons = ctx.enter_context(tc.tile_pool(name='cons', bufs=1))
        esp = ctx.enter_context(tc.tile_pool(name='esp', bufs=3))
        smalls = ctx.enter_context(tc.tile_pool(name='smalls', bufs=4))
        r64p = ctx.enter_context(tc.tile_pool(name='r64p', bufs=3))
        osb_p = ctx.enter_context(tc.tile_pool(name='osb', bufs=4))
        attnp = ctx.enter_context(tc.tile_pool(name='attnp', bufs=3))
        sc_ps = ctx.enter_context(tc.tile_pool(name='sc_ps', bufs=2, space='PSUM'))
        av_ps = ctx.enter_context(tc.tile_pool(name='av_ps', bufs=2, space='PSUM'))
        wo_ps = ctx.enter_context(tc.tile_pool(name='wo_ps', bufs=2, space='PSUM'))

        # PE warmup against the HAM clock-gate while input DMAs stream.
        wrm = cons.tile([128, 512], dt.bfloat16, name='wrm')
        nc.vector.memset(wrm, 0.0)
        wps = wo_ps.tile([128, QC], dt.float32, tag='wop')
        for _ in range(8):
            nc.tensor.matmul(wps, wrm[:, 0:128], wrm, start=True, stop=True)

        # DMA order: first 512 cols of K and Q (block c=0 work), small
        # constants, rest of K, Q cols 512:1024, w_o, then the rest of Q.
        kt_sb = cons.tile([128, S], dt.bfloat16)
        qt_sb = cons.tile([128, 4, S], dt.bfloat16)
        nc.sync.dma_start(out=kt_sb[:, 0:512], in_=kt2[:, 0:512])
        nc.sync.dma_start(out=qt_sb[:, :, 0:512], in_=qt2[:, :, 0:512])
        mc_sb = cons.tile([128, 4, 512], dt.bfloat16)
        nc.sync.dma_start(out=mc_sb, in_=mc4)
        i128_sb = cons.tile([128, 128], dt.bfloat16)
        nc.sync.dma_start(out=i128_sb, in_=i128)
        va_sb = cons.tile([128, 16, HD + 1], dt.bfloat16)
        nc.sync.dma_start(out=va_sb, in_=va2)
        nc.sync.dma_start(out=kt_sb[:, 512:S], in_=kt2[:, 512:S])
        nc.sync.dma_start(out=qt_sb[:, :, 512:1024], in_=qt2[:, :, 512:1024])
        wo_sb = cons.tile([128, 32, 2, 128], dt.bfloat16)
        nc.sync.dma_start(out=wo_sb, in_=wo2)
        nc.sync.dma_start(out=qt_sb[:, :, 1024:S], in_=qt2[:, :, 1024:S])

        attns = [None] * 4

        def att_chain(c, hl):
            nt = 4 * c + 4
            q0 = QC * c
            pr, ph = hl // 2, hl % 2
            avp = av_ps.tile([128, QC], dt.float32, tag='avp')
            for g0 in range(0, nt, 2):
                sp = sc_ps.tile([128, 2 * QC], dt.float32, tag='scp')
                masked = g0 >= nt - 4     # diagonal tiles: causal mask applies
                for i in range(2):
                    t = g0 + i
                    h0 = 64 * i
                    nc.tensor.matmul(sp[:, QC * i:QC * (i + 1)],
                                     kt_sb[h0:h0 + 64,
                                           128 * t:128 * (t + 1)],
                                     qt_sb[h0:h0 + 64, hl, q0:q0 + QC],
                                     start=True, stop=not masked)
                    if masked:
                        # accumulate -30 into masked score positions so exp
                        # yields ~0 there (mc holds -30*(1-causal_mask))
                        nc.tensor.matmul(sp[:, QC * i:QC * (i + 1)],
                                         i128_sb, mc_sb[:, t - (nt - 4), :],
                                         start=False, stop=True)
                es = esp.tile([128, 2 * QC], dt.bfloat16, tag='es')
                nc.scalar.activation(es, sp, Exp)
                for i in range(2):
                    t = g0 + i
                    nc.tensor.matmul(avp[0:HD + 1, :], va_sb[:, t, :],
                                     es[:, QC * i:QC * (i + 1)],
                                     start=(t == 0), stop=(t == nt - 1))
            zsb = smalls.tile([1, QC], dt.float32, tag='zsb')
            nc.vector.tensor_copy(zsb, avp[HD:HD + 1, :])
            rec = smalls.tile([1, QC], dt.float32, tag='rec')
            nc.vector.reciprocal_approx_fast(rec, zsb)
            r64 = r64p.tile([64, QC], dt.float32, tag='r64')
            nc.gpsimd.partition_broadcast(r64, rec)
            nc.vector.tensor_tensor(
                attns[c][64 * ph:64 * (ph + 1), pr, :],
                avp[0:HD, :], r64, op=Mult)

        def wo_slice(c, ocs, final=False):
            q0 = QC * c
            for oc in ocs:
                ps = wo_ps.tile([128, QC], dt.float32, tag='wop')
                for lt in range(2):
                    nc.tensor.matmul(ps, wo_sb[:, oc, lt, :],
                                     attns[c][:, lt, :],
                                     start=(lt == 0), stop=(lt == 1))
                ob = osb_p.tile([128, QC], dt.bfloat16, tag='ob')
                on_dve = (oc % 2 == 0) if final else (oc % 5 != 4)
                if on_dve:
                    nc.vector.tensor_copy(ob, ps)
                else:
                    nc.scalar.copy(ob, ps)
                nc.sync.dma_start(
                    out=po[128 * oc:128 * (oc + 1), q0:q0 + QC],
                    in_=ob)

        # software pipeline: w_o slices of block c-1 interleave with the
        # attention head-chains of block c, so the PE never waits long on
        # the exp/normalize chain and the DVE copy backlog never delays
        # the next attention block.
        for c in range(S // QC):
            attns[c] = attnp.tile([128, 2, QC], dt.bfloat16, tag='attn',
                                  name=f'attn{c}')
            for hl in range(4):
                att_chain(c, hl)
                if c >= 1:
                    wo_slice(c - 1, range(8 * hl, 8 * (hl + 1)))
        for hl in range(4):
            wo_slice(3, range(8 * hl, 8 * (hl + 1)), final=(hl == 3))

    nc.compile()
    return nc


# ---------------------------------------------------------------------------
# host-side data prep
# ---------------------------------------------------------------------------

def _chunk_starts(g):
    return (CH * g, S - CH * (g + 1))


def _prep_launch1(x, w_qkv, qc1_w, qc1_b, qc2_w, qc2_b, kc1_w, kc1_b,
                  kc2_w, kc2_b, key_temp):
    temp = float(np.asarray(key_temp).reshape(-1)[0])
    w_q = w_qkv[:LAT]
    w_k = w_qkv[LAT:LAT + KVD]
    w_v = w_qkv[LAT + KVD:]
    W_all = np.concatenate([w_q, w_k], 0)                   # [1280, DIM]

    # wqk blob [10, 128, 32, 128]: [ct, p, k, c] = W_all[128ct+c, 128k+p]
    wqk = np.ascontiguousarray(
        W_all.reshape(10, 128, 32, 128).transpose(0, 3, 2, 1)).astype(BF)
    # M128[p, i] = 1/16 if p%64 == i%64 (head-mean reduction matrix)
    idx128 = np.arange(128)
    m128 = ((idx128[:, None] % 64) == (idx128[None, :] % 64)).astype(F32) / 16.0
    m128 = m128.astype(BF)
    wv = np.zeros((128, 32, 256), BF)
    wvT = w_v.astype(BF)                                     # [256, DIM]
    for k in range(32):
        wv[:, k, :] = wvT[:, 128 * k:128 * (k + 1)].T

    def c1blob(w, npairs):
        out = np.zeros((128, 3, npairs, 128), F32)
        for j in range(KSZ):
            for p in range(npairs):
                for hh in range(2):
                    blk = w[HD * (2 * p + hh):HD * (2 * p + hh + 1), :, j]
                    out[HD * hh:HD * (hh + 1), j, p,
                        HD * hh:HD * (hh + 1)] = blk.T
        return out.astype(BF)

    qc1 = c1blob(qc1_w, 8)
    kc1 = c1blob(kc1_w, 2)

    # qc2 blob [8, 128, 3, 8, 128]: [oc, p, j, it, c] = qc2_w[128oc+c, 128it+p, j]
    qc2 = np.ascontiguousarray(
        qc2_w.reshape(8, 128, 8, 128, 3).transpose(0, 3, 4, 2, 1)).astype(BF)
    kc2 = np.ascontiguousarray(
        kc2_w.reshape(2, 128, 2, 128, 3).transpose(0, 3, 4, 2, 1)).astype(BF)

    inv = 1.0 / (10000.0 ** (np.arange(0, HD, 2, dtype=F32) / HD))
    ang = np.arange(S, dtype=F32)[:, None] * inv[None, :]    # [S, 32]
    cosT, sinT = np.cos(ang), np.sin(ang)
    sgn = np.where(np.arange(HD) % 2 == 0, -1.0, 1.0).astype(F32)

    def cs_blob(g, scale):
        out = np.zeros((128, 2, 2, CH), F32)
        for ch, q0 in enumerate(_chunk_starts(g)):
            pos = np.arange(q0, q0 + CH)
            c = np.repeat(cosT[pos], 2, axis=1).T * scale    # [64, CH]
            s = np.repeat(sinT[pos], 2, axis=1).T * scale * sgn[:, None]
            out[:, ch, 0, :] = np.tile(c, (2, 1))
            out[:, ch, 1, :] = np.tile(s, (2, 1))
        return out

    e2 = np.zeros((128, 2), F32)
    e2[0:64, 0] = 1.0
    e2[64:128, 1] = 1.0
    e2t = np.ascontiguousarray(e2.T)
    p128 = np.zeros((128, 128), F32)
    idx = np.arange(128)
    p128[idx, idx ^ 1] = 1.0

    shared = dict(
        wqk=wqk, wv=wv, qc1=qc1, kc1=kc1, qc2=qc2, kc2=kc2, m128=m128,
        e2=e2.astype(F16), e2t=e2t.astype(F16), p128=p128.astype(F16),
        qc1b=np.ascontiguousarray(qc1_b.reshape(8, 128).T).astype(F32),
        kc1b=np.ascontiguousarray(kc1_b.reshape(2, 128).T).astype(F32),
        qc2b=np.ascontiguousarray(qc2_b.reshape(8, 128).T).astype(F32),
        kc2b=np.ascontiguousarray(kc2_b.reshape(2, 128).T).astype(F32),
    )

    x_bf = x.astype(BF)
    scale_q = 1.0 / np.sqrt(HD)
    in_maps = []
    for core in range(NCORES):
        bb, g = core // 4, core % 4
        xtb = np.zeros((128, 32, 2 * CW), BF)
        for ch, q0 in enumerate(_chunk_starts(g)):
            rows = np.arange(q0 - HALO, q0 + CH)
            xr = x_bf[bb, np.clip(rows, 0, None)]            # [260, DIM]
            if rows[0] < 0:
                xr = xr.copy()
                xr[rows < 0] = 0
            xrT = xr.T.reshape(32, 128, CW).transpose(1, 0, 2)
            xtb[:, :, ch * CW:(ch + 1) * CW] = xrT
        m = dict(shared)
        m['xt'] = xtb
        m['csq'] = cs_blob(g, scale_q).astype(F16)
        m['csk'] = cs_blob(g, temp).astype(F16)
        in_maps.append(m)
    return in_maps


def _prep_launch2(res1, w_o):
    # res1: list of 8 dicts with qat [1024,512], kat [256,512], vo [512,256]
    QT = np.zeros((B, LAT, S), BF)
    KT = np.zeros((B, KVD, S), BF)
    V = np.zeros((B, S, KVD), BF)
    for core in range(NCORES):
        bb, g = core // 4, core % 4
        r = res1[core]
        for ch, q0 in enumerate(_chunk_starts(g)):
            QT[bb, :, q0:q0 + CH] = r['qat'][:, CH * ch:CH * (ch + 1)]
            KT[bb, :, q0:q0 + CH] = r['kat'][:, CH * ch:CH * (ch + 1)]
            V[bb, q0:q0 + CH] = r['vo'][CH * ch:CH * (ch + 1)]

    mc4 = np.zeros((128, 4, 512), np.float32)
    for k in range(4):
        keep = (128 * k + np.arange(128)[:, None]
                <= np.arange(512)[None, :])
        mc4[:, k, :] = -30.0 * (1.0 - keep)
    mc4 = mc4.astype(BF)
    i128 = np.eye(128, dtype=np.float32).astype(BF)

    # wo blobs per kvh: [128, 32, 2, 128]
    wo_blobs = []
    for kvh in range(NKV):
        blk = w_o[:, KVD * kvh:KVD * (kvh + 1)].astype(BF)   # [4096, 256]
        wo_blobs.append(np.ascontiguousarray(
            blk.reshape(32, 128, 2, 128).transpose(3, 0, 2, 1)))
        # [p, oc, lt, c] = blk[128oc+c, 128lt+p]

    in_maps = []
    for core in range(NCORES):
        bb, kvh = core // 4, core % 4
        Vsh = np.zeros((S, HD + 1), BF)
        Vsrc = V[bb]
        base = HD * kvh
        if base + HD <= KVD // 2:
            Vsh[:, :HD] = Vsrc[:, base:base + HD]
        else:
            Vsh[1:, :HD] = Vsrc[:-1, base:base + HD]
        Vsh[:, HD] = 1.0
        va2 = np.ascontiguousarray(
            Vsh.reshape(16, 128, HD + 1).transpose(1, 0, 2))

        qt2 = np.zeros((128, 4, S), BF)
        for hl in range(4):
            h = 4 * kvh + hl
            qt2[0:64, hl, :] = QT[bb, HD * h:HD * (h + 1)]
            qt2[64:128, hl, :] = QT[bb, HD * h:HD * (h + 1)]
        kt2 = np.zeros((128, S), BF)
        kt2[0:64] = KT[bb, HD * kvh:HD * (kvh + 1), :]
        kt2[64:128] = kt2[0:64]
        in_maps.append(dict(qt2=qt2, kt2=kt2, va2=va2, wo2=wo_blobs[kvh],
                            mc4=mc4, i128=i128))
    return in_maps


# ---------------------------------------------------------------------------
# entry point
# ---------------------------------------------------------------------------

def _get_progs(with_qc2b, with_kc2b):
    key = (with_qc2b, with_kc2b)
    if key not in _PROGS:
        _PROGS[key] = (_build_launch1(with_qc2b, with_kc2b), _build_launch2())
    return _PROGS[key]


def _run(nc, in_maps, **kw):
    from concourse.bass_utils import run_bass_kernel_spmd
    return run_bass_kernel_spmd(nc, in_maps, list(range(NCORES)), **kw)


def kernel(x, w_qkv, w_o, qc1_w, qc1_b, qc2_w, qc2_b, kc1_w, kc1_b,
           kc2_w, kc2_b, key_temp, _profile=False):
    args = [np.asarray(a, F32) for a in
            (x, w_qkv, w_o, qc1_w, qc1_b, qc2_w, qc2_b, kc1_w, kc1_b,
             kc2_w, kc2_b, key_temp)]
    (x, w_qkv, w_o, qc1_w, qc1_b, qc2_w, qc2_b, kc1_w, kc1_b,
     kc2_w, kc2_b, key_temp) = args

    with_qc2b = bool(np.any(qc2_b))
    with_kc2b = bool(np.any(kc2_b))
    nc1, nc2 = _get_progs(with_qc2b, with_kc2b)

    maps1 = _prep_launch1(x, w_qkv, qc1_w, qc1_b, qc2_w, qc2_b,
                          kc1_w, kc1_b, kc2_w, kc2_b, key_temp)
    kw = dict(trace=True) if _profile else {}
    r1 = _run(nc1, maps1, **kw)
    kernel.exec_ns_1 = r1.exec_time_ns
    maps2 = _prep_launch2(r1.results, w_o)
    r2 = _run(nc2, maps2, **kw)
    kernel.exec_ns_2 = r2.exec_time_ns

    out = np.zeros((B, S, DIM), F32)
    for bb in range(B):
        acc = np.zeros((DIM, S), F32)
        for kvh in range(NKV):
            acc += r2.results[4 * bb + kvh]['po'].astype(F32)
        out[bb] = acc.T
    return out



# revision 43
# speedup vs baseline: 1.0812x; 1.0812x over previous
"""CCGQA attention on 8 Trainium2 NeuronCores (Bass/Tile, SPMD).

Decomposition (uniform SPMD programs; per-core differences are data only):
  Launch 1 - core (b, g), g in 0..3: preprocessing for two folded 256-row
    query chunks {g, 7-g} of batch b (plus 4-row conv halos):
    fused QKV+head-mean GEMM, grouped conv1 + dense conv2 (shifted matmuls),
    qk-mean coupling, l2-norm (partition reduction via matmul with a
    block-ones matrix), RoPE (pair swap via permutation matmul; softmax
    1/sqrt(d) folded into q cos/sin tables, key_temp folded into k tables),
    and the V projection in row-major layout.
  Host: reassemble full K^T, Q^T, V per batch, apply the v_prev shift,
    append a ones column to V (so the attn@V matmul chain also produces the
    softmax denominator; qk-norm bounds scores to [-1,1] so exp needs no
    max subtraction).
  Launch 2 - core (b, kvh): causal attention for 4 q heads / 1 kv head over
    all 2048 rows + row-split w_o partials (bf16), summed on host.
"""
import sys
import os

sys.path.insert(0, '/opt/trn_rl_repo')

import numpy as np
import ml_dtypes

BF = ml_dtypes.bfloat16
F16 = np.float16
F32 = np.float32

B, S, DIM = 2, 2048, 4096
NH, NKV, HD = 16, 4, 64
LAT, KVD, KSZ = 1024, 256, 3
CH = 256            # fold chunk rows
HALO = 4
CW = CH + HALO      # 260 cols per chunk (with halo)
NCORES = 8

_PROGS = {}


def _dt():
    from concourse import mybir
    return mybir.dt


# ---------------------------------------------------------------------------
# program builders
# ---------------------------------------------------------------------------

def _build_launch1(with_qc2b, with_kc2b, phases=('gemm1','v','conv2','tail')):
    from contextlib import ExitStack
    import concourse.tile as tile
    from concourse import bacc, mybir

    dt = mybir.dt
    nc = bacc.Bacc('TRN2', target_bir_lowering=False, debug=False,
                   num_devices=NCORES)

    def din(name, shape, dtype=dt.bfloat16):
        return nc.dram_tensor(name, shape, dtype, kind='ExternalInput').ap()

    def dout(name, shape, dtype=dt.bfloat16):
        return nc.dram_tensor(name, shape, dtype, kind='ExternalOutput').ap()

    xt = din('xt', [128, 32, 2 * CW])
    wqk = din('wqk', [10, 128, 32, 128])
    m128 = din('m128', [128, 128])
    wv = din('wv', [128, 32, 256])
    qc1 = din('qc1', [128, 3, 8, 128])
    kc1 = din('kc1', [128, 3, 2, 128])
    qc2 = din('qc2', [8, 128, 3, 8, 128])
    kc2 = din('kc2', [2, 128, 3, 2, 128])
    csq = din('csq', [128, 2, 2, CH], dt.float16)
    csk = din('csk', [128, 2, 2, CH], dt.float16)
    e2 = din('e2', [128, 2], dt.float16)
    e2t = din('e2t', [2, 128], dt.float16)
    p128 = din('p128', [128, 128], dt.float16)
    qc1b = din('qc1b', [128, 8], dt.float32)
    kc1b = din('kc1b', [128, 2], dt.float32)
    qc2b = din('qc2b', [128, 8], dt.float32)
    kc2b = din('kc2b', [128, 2], dt.float32)

    qat = dout('qat', [LAT, 2 * CH])
    kat = dout('kat', [KVD, 2 * CH])
    vo = dout('vo', [2 * CH, KVD])

    Add = mybir.AluOpType.add
    Mult = mybir.AluOpType.mult
    Sqrt = mybir.ActivationFunctionType.Sqrt

    with tile.TileContext(nc) as tc, ExitStack() as ctx:
        cons = ctx.enter_context(tc.tile_pool(name='cons', bufs=1))
        wpool = ctx.enter_context(tc.tile_pool(name='wpool', bufs=3))
        qpre = ctx.enter_context(tc.tile_pool(name='qpre', bufs=3))
        c1buf = ctx.enter_context(tc.tile_pool(name='c1buf', bufs=1))
        tmp = ctx.enter_context(tc.tile_pool(name='tmp', bufs=4))
        small = ctx.enter_context(tc.tile_pool(name='small', bufs=4))
        outp = ctx.enter_context(tc.tile_pool(name='outp', bufs=3))
        gemm_ps = ctx.enter_context(tc.tile_pool(name='gemm_ps', bufs=2, space='PSUM'))
        mean_ps = ctx.enter_context(tc.tile_pool(name='mean_ps', bufs=1, space='PSUM'))
        conv_ps = ctx.enter_context(tc.tile_pool(name='conv_ps', bufs=2, space='PSUM'))
        nrm_ps = ctx.enter_context(tc.tile_pool(name='nrm_ps', bufs=1, space='PSUM'))

        # DMA order tuned so the first GEMM matmuls start ASAP: first weight
        # tile, then x pieces in consumption order, then the second weight
        # tile, then everything else.
        # PE warmup: dummy matmuls on a memset tile so the HAM clock-gate is
        # released (and stays released) before the first real matmul.
        wrm = cons.tile([128, 512], dt.bfloat16, name='wrm')
        nc.vector.memset(wrm, 0.0)
        wps = gemm_ps.tile([128, 512], dt.float32, tag='gps')
        for _ in range(8):
            nc.tensor.matmul(wps, wrm[:, 0:128], wrm, start=True, stop=True)

        w01 = []
        for ct in range(2):
            w = wpool.tile([128, 32, 128], dt.bfloat16, tag='wstream')
            w01.append(w)
        xt_sbs = [cons.tile([128, 8, 2 * CW], dt.bfloat16, name=f'xt_sb{xq}')
                  for xq in range(4)]
        nc.sync.dma_start(out=w01[0][:, 0:16, :], in_=wqk[0][:, 0:16, :])
        nc.sync.dma_start(out=xt_sbs[0][:, :, 0:CW], in_=xt[:, 0:8, 0:CW])
        nc.sync.dma_start(out=xt_sbs[1][:, :, 0:CW], in_=xt[:, 8:16, 0:CW])
        nc.sync.dma_start(out=w01[0][:, 16:32, :], in_=wqk[0][:, 16:32, :])
        nc.sync.dma_start(out=xt_sbs[2][:, :, 0:CW], in_=xt[:, 16:24, 0:CW])
        nc.sync.dma_start(out=xt_sbs[3][:, :, 0:CW], in_=xt[:, 24:32, 0:CW])
        nc.sync.dma_start(out=w01[1], in_=wqk[1])
        for xq in range(4):
            nc.sync.dma_start(out=xt_sbs[xq][:, :, CW:2 * CW],
                              in_=xt[:, 8 * xq:8 * (xq + 1), CW:2 * CW])
        qc1_sb = cons.tile([128, 3, 8, 128], dt.bfloat16)
        nc.sync.dma_start(out=qc1_sb, in_=qc1)
        kc1_sb = cons.tile([128, 3, 2, 128], dt.bfloat16)
        nc.sync.dma_start(out=kc1_sb, in_=kc1)
        qc1b_sb = cons.tile([128, 8], dt.float32)
        nc.sync.dma_start(out=qc1b_sb, in_=qc1b)
        kc1b_sb = cons.tile([128, 2], dt.float32)
        nc.sync.dma_start(out=kc1b_sb, in_=kc1b)
        m128_sb = cons.tile([128, 128], dt.bfloat16)
        nc.sync.dma_start(out=m128_sb, in_=m128)

        # persistent intermediates
        c1q = c1buf.tile([128, 2, 8, CH + 2], dt.bfloat16)   # conv1(q) both chunks
        c1k = c1buf.tile([128, 2, 2, CH + 2], dt.bfloat16)
        qmd = c1buf.tile([128, 2, CH], dt.float32)           # qmean/16 duplicated
        kmd = c1buf.tile([128, 2, CH], dt.float32)           # kmean/4  duplicated

        # head means accumulate in PSUM via matmuls against M128 (1/16 block
        # matrix) as each pre tile is produced; copied to SBUF afterwards.
        qm_ps = mean_ps.tile([128, 2, CH], dt.float32, tag='qmps')
        km_ps = mean_ps.tile([128, 2, CH], dt.float32, tag='kmps')

        # ---- GEMM1 (qk) + head means + grouped conv1 ----
        for ct in range(10):
            if ct < 2:
                w = w01[ct]
            else:
                w = wpool.tile([128, 32, 128], dt.bfloat16, tag='wstream')
                nc.sync.dma_start(out=w, in_=wqk[ct])
            pre = qpre.tile([128, 2, CW], dt.bfloat16, tag='pre')
            for ch in range(2):
                ps = gemm_ps.tile([128, CW], dt.float32, tag='gps')
                for kt in range(32):
                    nc.tensor.matmul(ps, w[:, kt, :],
                                     xt_sbs[kt // 8][:, kt % 8,
                                                     ch * CW:(ch + 1) * CW],
                                     start=(kt == 0), stop=(kt == 31))
                nc.vector.tensor_copy(pre[:, ch, :], ps)
                # conv1 for this tile (pair of heads)
                c1ps = conv_ps.tile([128, CH + 2], dt.float32, tag='cps')
                if ct < 8:
                    for j in range(3):
                        nc.tensor.matmul(c1ps, qc1_sb[:, j, ct, :],
                                         pre[:, ch, j:j + CH + 2],
                                         start=(j == 0), stop=(j == 2))
                    nc.vector.tensor_scalar(c1q[:, ch, ct, :], c1ps,
                                            qc1b_sb[:, ct:ct + 1], None, Add)
                else:
                    p = ct - 8
                    for j in range(3):
                        nc.tensor.matmul(c1ps, kc1_sb[:, j, p, :],
                                         pre[:, ch, j:j + CH + 2],
                                         start=(j == 0), stop=(j == 2))
                    nc.vector.tensor_scalar(c1k[:, ch, p, :], c1ps,
                                            kc1b_sb[:, p:p + 1], None, Add)
            # single open accumulation group per PSUM bank: one 2D-AP mean
            # matmul per ct covering both chunks
            if ct < 8:
                nc.tensor.matmul(qm_ps, m128_sb, pre[:, :, HALO:CW],
                                 start=(ct == 0), stop=(ct == 7))
            else:
                nc.tensor.matmul(km_ps, m128_sb, pre[:, :, HALO:CW],
                                 start=(ct == 8), stop=(ct == 9))
        nc.vector.tensor_copy(qmd, qm_ps)
        nc.vector.tensor_copy(kmd, km_ps)

        # late constants (needed from conv2-tail onward)
        csq_sb = cons.tile([128, 2, 2, CH], dt.float16)
        nc.sync.dma_start(out=csq_sb, in_=csq)
        csk_sb = cons.tile([128, 2, 2, CH], dt.float16)
        nc.sync.dma_start(out=csk_sb, in_=csk)
        e2_sb = cons.tile([128, 2], dt.float16)
        nc.sync.dma_start(out=e2_sb, in_=e2)
        e2t_sb = cons.tile([2, 128], dt.float16)
        nc.sync.dma_start(out=e2t_sb, in_=e2t)
        p128_sb = cons.tile([128, 128], dt.float16)
        nc.sync.dma_start(out=p128_sb, in_=p128)
        qc2b_sb = cons.tile([128, 8], dt.float32)
        nc.sync.dma_start(out=qc2b_sb, in_=qc2b)
        kc2b_sb = cons.tile([128, 2], dt.float32)
        nc.sync.dma_start(out=kc2b_sb, in_=kc2b)
        eps_sb = cons.tile([128, 1], dt.float32)
        nc.vector.memset(eps_sb, 1e-24)

        # ---- V projection (row-major) ----
        if 'v' in phases:
            wv_sb = cons.tile([128, 32, 256], dt.bfloat16)
            nc.sync.dma_start(out=wv_sb, in_=wv)
        for rt in range(4 if 'v' in phases else 0):
            c0 = HALO + 128 * rt if rt < 2 else CW + HALO + 128 * (rt - 2)
            ps = gemm_ps.tile([128, CW], dt.float32, tag='gps')
            for kt in range(32):
                nc.tensor.matmul(ps[:, 0:256],
                                 xt_sbs[kt // 8][:, kt % 8, c0:c0 + 128],
                                 wv_sb[:, kt, :],
                                 start=(kt == 0), stop=(kt == 31))
            vsb = outp.tile([128, 256], dt.bfloat16, tag='vout')
            nc.vector.tensor_copy(vsb, ps[:, 0:256])
            nc.sync.dma_start(out=vo[128 * rt:128 * (rt + 1), :], in_=vsb)

        # ---- conv2 + coupling + l2norm + rope ----
        def tail(oc, ch, ps2, mdup, coeff, bias_sb, with_bias, cs_sb, out_dram):
            qf = tmp.tile([128, CH], dt.float16, tag='qf')
            nc.vector.scalar_tensor_tensor(qf, mdup[:, ch, :], coeff, ps2,
                                           op0=Mult, op1=Add)
            if with_bias:
                nc.vector.tensor_scalar(qf, qf, bias_sb[:, oc:oc + 1], None, Add)
            sq = tmp.tile([128, CH], dt.float16, tag='sq')
            nc.vector.tensor_tensor(sq, qf, qf, op=Mult)
            nps = nrm_ps.tile([128, CH], dt.float32, tag='nps')
            nc.tensor.matmul(nps[0:2, :], e2_sb, sq, start=True, stop=True)
            nrm = small.tile([2, CH], dt.float32, tag='nrm')
            nc.scalar.activation(nrm, nps[0:2, :], Sqrt, bias=eps_sb[0:2, :])
            rin = small.tile([2, CH], dt.float32, tag='rin')
            nc.vector.reciprocal_approx_fast(rin, nrm)
            rin16 = small.tile([2, CH], dt.float16, tag='rin16')
            nc.scalar.copy(rin16, rin)
            bs = nrm_ps.tile([128, 2, CH], dt.float32, tag='bs')
            bps = bs[:, 0, :]
            nc.tensor.matmul(bps, e2t_sb, rin16, start=True, stop=True)
            sps = bs[:, 1, :]
            nc.tensor.matmul(sps, p128_sb, qf, start=True, stop=True)
            t1 = tmp.tile([128, CH], dt.float16, tag='t1')
            nc.vector.tensor_tensor(t1, qf, cs_sb[:, ch, 0, :], op=Mult)
            t2 = tmp.tile([128, CH], dt.float16, tag='t2')
            nc.vector.tensor_tensor(t2, sps, cs_sb[:, ch, 1, :], op=Mult)
            t3 = tmp.tile([128, CH], dt.float16, tag='t3')
            nc.vector.tensor_add(t3, t1, t2)
            qo = outp.tile([128, CH], dt.bfloat16, tag='qo')
            nc.vector.tensor_tensor(qo, t3, bps, op=Mult)
            nc.sync.dma_start(
                out=out_dram[128 * oc:128 * (oc + 1), CH * ch:CH * (ch + 1)],
                in_=qo)

        # conv2 q/k interleaved so the small k-convs' tails overlap with
        # later q-conv matmuls instead of serializing at the very end.
        seq = [('q', 0), ('q', 1), ('q', 2), ('q', 3), ('k', 0),
               ('q', 4), ('q', 5), ('k', 1), ('q', 6), ('q', 7)]
        for kind, oc in (seq if 'conv2' in phases else []):
            nit = 8 if kind == 'q' else 2
            w2 = wpool.tile([128, 3, nit, 128], dt.bfloat16, tag='wstream')
            nc.sync.dma_start(out=w2, in_=(qc2[oc] if kind == 'q' else kc2[oc]))
            c1 = c1q if kind == 'q' else c1k
            for ch in range(2):
                ps2 = conv_ps.tile([128, CH + 2], dt.float32, tag='cps')
                n = 0
                for j in range(3):
                    for it in range(nit):
                        nc.tensor.matmul(ps2[:, 0:CH], w2[:, j, it, :],
                                         c1[:, ch, it, j:j + CH],
                                         start=(n == 0), stop=(n == 3 * nit - 1))
                        n += 1
                if 'tail' in phases:
                    if kind == 'q':
                        tail(oc, ch, ps2[:, 0:CH], kmd, 2.0, qc2b_sb,
                             with_qc2b, csq_sb, qat)
                    else:
                        tail(oc, ch, ps2[:, 0:CH], qmd, 0.5, kc2b_sb,
                             with_kc2b, csk_sb, kat)
                else:
                    qo = outp.tile([128, CH], dt.bfloat16, tag='qo')
                    nc.vector.tensor_copy(qo, ps2[:, 0:CH])
                    od = qat if kind == 'q' else kat
                    nc.sync.dma_start(
                        out=od[128 * oc:128 * (oc + 1), CH * ch:CH * (ch + 1)],
                        in_=qo)

    nc.compile()
    return nc


def _build_launch2():
    from contextlib import ExitStack
    import concourse.tile as tile
    from concourse import bacc, mybir

    dt = mybir.dt
    nc = bacc.Bacc('TRN2', target_bir_lowering=False, debug=False,
                   num_devices=NCORES)

    def din(name, shape, dtype=dt.bfloat16):
        return nc.dram_tensor(name, shape, dtype, kind='ExternalInput').ap()

    qt2 = din('qt2', [128, 4, S])
    kt2 = din('kt2', [128, S])
    va2 = din('va2', [128, 16, HD + 1])
    wo2 = din('wo2', [128, 32, 2, 128])
    mc4 = din('mc4', [128, 4, 512])
    po = nc.dram_tensor('po', [DIM, S], dt.bfloat16, kind='ExternalOutput').ap()

    Mult = mybir.AluOpType.mult
    Exp = mybir.ActivationFunctionType.Exp
    Log = mybir.ActivationFunctionType.Ln
    QC = 512                      # query chunk width

    with tile.TileContext(nc) as tc, ExitStack() as ctx:
        cons = ctx.enter_context(tc.tile_pool(name='cons', bufs=1))
        esp = ctx.enter_context(tc.tile_pool(name='esp', bufs=3))
        smalls = ctx.enter_context(tc.tile_pool(name='smalls', bufs=4))
        r64p = ctx.enter_context(tc.tile_pool(name='r64p', bufs=3))
        osb_p = ctx.enter_context(tc.tile_pool(name='osb', bufs=4))
        attnp = ctx.enter_context(tc.tile_pool(name='attnp', bufs=3))
        sc_ps = ctx.enter_context(tc.tile_pool(name='sc_ps', bufs=2, space='PSUM'))
        av_ps = ctx.enter_context(tc.tile_pool(name='av_ps', bufs=2, space='PSUM'))
        wo_ps = ctx.enter_context(tc.tile_pool(name='wo_ps', bufs=2, space='PSUM'))

        # PE warmup against the HAM clock-gate while input DMAs stream.
        wrm = cons.tile([128, 512], dt.bfloat16, name='wrm')
        nc.vector.memset(wrm, 0.0)
        wps = wo_ps.tile([128, QC], dt.float32, tag='wop')
        for _ in range(8):
            nc.tensor.matmul(wps, wrm[:, 0:128], wrm, start=True, stop=True)

        # DMA order: first 512 cols of K and Q (block c=0 work), small
        # constants, rest of K, Q cols 512:1024, w_o, then the rest of Q.
        kt_sb = cons.tile([128, S], dt.bfloat16)
        qt_sb = cons.tile([128, 4, S], dt.bfloat16)
        nc.sync.dma_start(out=kt_sb[:, 0:512], in_=kt2[:, 0:512])
        nc.sync.dma_start(out=qt_sb[:, :, 0:512], in_=qt2[:, :, 0:512])
        mc_sb = cons.tile([128, 4, 512], dt.bfloat16)
        nc.sync.dma_start(out=mc_sb, in_=mc4)
        va_sb = cons.tile([128, 16, HD + 1], dt.bfloat16)
        nc.sync.dma_start(out=va_sb, in_=va2)
        nc.sync.dma_start(out=kt_sb[:, 512:S], in_=kt2[:, 512:S])
        nc.sync.dma_start(out=qt_sb[:, :, 512:1024], in_=qt2[:, :, 512:1024])
        wo_sb = cons.tile([128, 32, 2, 128], dt.bfloat16)
        nc.sync.dma_start(out=wo_sb, in_=wo2)
        nc.sync.dma_start(out=qt_sb[:, :, 1024:S], in_=qt2[:, :, 1024:S])

        attns = [None] * 4

        def att_chain(c, hl):
            nt = 4 * c + 4
            q0 = QC * c
            pr, ph = hl // 2, hl % 2
            avp = av_ps.tile([128, QC], dt.float32, tag='avp')
            for g0 in range(0, nt, 2):
                sp = sc_ps.tile([128, 2 * QC], dt.float32, tag='scp')
                for i in range(2):
                    t = g0 + i
                    h0 = 64 * i
                    nc.tensor.matmul(sp[:, QC * i:QC * (i + 1)],
                                     kt_sb[h0:h0 + 64,
                                           128 * t:128 * (t + 1)],
                                     qt_sb[h0:h0 + 64, hl, q0:q0 + QC],
                                     start=True, stop=True)
                es = esp.tile([128, 2 * QC], dt.bfloat16, tag='es')
                nc.scalar.activation(es, sp, Exp)
                if g0 == nt - 4:
                    nc.vector.tensor_tensor(
                        es, es, mc_sb[:, 0:2, :].rearrange('p a b -> p (a b)'),
                        op=Mult)
                elif g0 == nt - 2:
                    nc.vector.tensor_tensor(
                        es, es, mc_sb[:, 2:4, :].rearrange('p a b -> p (a b)'),
                        op=Mult)
                for i in range(2):
                    t = g0 + i
                    nc.tensor.matmul(avp[0:HD + 1, :], va_sb[:, t, :],
                                     es[:, QC * i:QC * (i + 1)],
                                     start=(t == 0), stop=(t == nt - 1))
            zsb = smalls.tile([1, QC], dt.float32, tag='zsb')
            nc.vector.tensor_copy(zsb, avp[HD:HD + 1, :])
            rec = smalls.tile([1, QC], dt.float32, tag='rec')
            nc.vector.reciprocal_approx_fast(rec, zsb)
            r64 = r64p.tile([64, QC], dt.float32, tag='r64')
            nc.gpsimd.partition_broadcast(r64, rec)
            nc.vector.tensor_tensor(
                attns[c][64 * ph:64 * (ph + 1), pr, :],
                avp[0:HD, :], r64, op=Mult)

        def wo_slice(c, ocs, final=False):
            q0 = QC * c
            for oc in ocs:
                ps = wo_ps.tile([128, QC], dt.float32, tag='wop')
                for lt in range(2):
                    nc.tensor.matmul(ps, wo_sb[:, oc, lt, :],
                                     attns[c][:, lt, :],
                                     start=(lt == 0), stop=(lt == 1))
                ob = osb_p.tile([128, QC], dt.bfloat16, tag='ob')
                on_dve = (oc % 2 == 0) if final else (oc % 3 != 2)
                if on_dve:
                    nc.vector.tensor_copy(ob, ps)
                else:
                    nc.scalar.copy(ob, ps)
                nc.sync.dma_start(
                    out=po[128 * oc:128 * (oc + 1), q0:q0 + QC],
                    in_=ob)

        # software pipeline: w_o slices of block c-1 interleave with the
        # attention head-chains of block c, so the PE never waits long on
        # the exp/normalize chain and the DVE copy backlog never delays
        # the next attention block.
        for c in range(S // QC):
            attns[c] = attnp.tile([128, 2, QC], dt.bfloat16, tag='attn',
                                  name=f'attn{c}')
            for hl in range(4):
                att_chain(c, hl)
                if c >= 1:
                    wo_slice(c - 1, range(8 * hl, 8 * (hl + 1)))
        for hl in range(4):
            wo_slice(3, range(8 * hl, 8 * (hl + 1)), final=(hl == 3))

    nc.compile()
    return nc


# ---------------------------------------------------------------------------
# host-side data prep
# ---------------------------------------------------------------------------

def _chunk_starts(g):
    return (CH * g, S - CH * (g + 1))


def _prep_launch1(x, w_qkv, qc1_w, qc1_b, qc2_w, qc2_b, kc1_w, kc1_b,
                  kc2_w, kc2_b, key_temp):
    temp = float(np.asarray(key_temp).reshape(-1)[0])
    w_q = w_qkv[:LAT]
    w_k = w_qkv[LAT:LAT + KVD]
    w_v = w_qkv[LAT + KVD:]
    W_all = np.concatenate([w_q, w_k], 0)                   # [1280, DIM]

    # wqk blob [10, 128, 32, 128]: [ct, p, k, c] = W_all[128ct+c, 128k+p]
    wqk = np.ascontiguousarray(
        W_all.reshape(10, 128, 32, 128).transpose(0, 3, 2, 1)).astype(BF)
    # M128[p, i] = 1/16 if p%64 == i%64 (head-mean reduction matrix)
    idx128 = np.arange(128)
    m128 = ((idx128[:, None] % 64) == (idx128[None, :] % 64)).astype(F32) / 16.0
    m128 = m128.astype(BF)
    wv = np.zeros((128, 32, 256), BF)
    wvT = w_v.astype(BF)                                     # [256, DIM]
    for k in range(32):
        wv[:, k, :] = wvT[:, 128 * k:128 * (k + 1)].T

    def c1blob(w, npairs):
        out = np.zeros((128, 3, npairs, 128), F32)
        for j in range(KSZ):
            for p in range(npairs):
                for hh in range(2):
                    blk = w[HD * (2 * p + hh):HD * (2 * p + hh + 1), :, j]
                    out[HD * hh:HD * (hh + 1), j, p,
                        HD * hh:HD * (hh + 1)] = blk.T
        return out.astype(BF)

    qc1 = c1blob(qc1_w, 8)
    kc1 = c1blob(kc1_w, 2)

    # qc2 blob [8, 128, 3, 8, 128]: [oc, p, j, it, c] = qc2_w[128oc+c, 128it+p, j]
    qc2 = np.ascontiguousarray(
        qc2_w.reshape(8, 128, 8, 128, 3).transpose(0, 3, 4, 2, 1)).astype(BF)
    kc2 = np.ascontiguousarray(
        kc2_w.reshape(2, 128, 2, 128, 3).transpose(0, 3, 4, 2, 1)).astype(BF)

    inv = 1.0 / (10000.0 ** (np.arange(0, HD, 2, dtype=F32) / HD))
    ang = np.arange(S, dtype=F32)[:, None] * inv[None, :]    # [S, 32]
    cosT, sinT = np.cos(ang), np.sin(ang)
    sgn = np.where(np.arange(HD) % 2 == 0, -1.0, 1.0).astype(F32)

    def cs_blob(g, scale):
        out = np.zeros((128, 2, 2, CH), F32)
        for ch, q0 in enumerate(_chunk_starts(g)):
            pos = np.arange(q0, q0 + CH)
            c = np.repeat(cosT[pos], 2, axis=1).T * scale    # [64, CH]
            s = np.repeat(sinT[pos], 2, axis=1).T * scale * sgn[:, None]
            out[:, ch, 0, :] = np.tile(c, (2, 1))
            out[:, ch, 1, :] = np.tile(s, (2, 1))
        return out

    e2 = np.zeros((128, 2), F32)
    e2[0:64, 0] = 1.0
    e2[64:128, 1] = 1.0
    e2t = np.ascontiguousarray(e2.T)
    p128 = np.zeros((128, 128), F32)
    idx = np.arange(128)
    p128[idx, idx ^ 1] = 1.0

    shared = dict(
        wqk=wqk, wv=wv, qc1=qc1, kc1=kc1, qc2=qc2, kc2=kc2, m128=m128,
        e2=e2.astype(F16), e2t=e2t.astype(F16), p128=p128.astype(F16),
        qc1b=np.ascontiguousarray(qc1_b.reshape(8, 128).T).astype(F32),
        kc1b=np.ascontiguousarray(kc1_b.reshape(2, 128).T).astype(F32),
        qc2b=np.ascontiguousarray(qc2_b.reshape(8, 128).T).astype(F32),
        kc2b=np.ascontiguousarray(kc2_b.reshape(2, 128).T).astype(F32),
    )

    x_bf = x.astype(BF)
    scale_q = 1.0 / np.sqrt(HD)
    in_maps = []
    for core in range(NCORES):
        bb, g = core // 4, core % 4
        xtb = np.zeros((128, 32, 2 * CW), BF)
        for ch, q0 in enumerate(_chunk_starts(g)):
            rows = np.arange(q0 - HALO, q0 + CH)
            xr = x_bf[bb, np.clip(rows, 0, None)]            # [260, DIM]
            if rows[0] < 0:
                xr = xr.copy()
                xr[rows < 0] = 0
            xrT = xr.T.reshape(32, 128, CW).transpose(1, 0, 2)
            xtb[:, :, ch * CW:(ch + 1) * CW] = xrT
        m = dict(shared)
        m['xt'] = xtb
        m['csq'] = cs_blob(g, scale_q).astype(F16)
        m['csk'] = cs_blob(g, temp).astype(F16)
        in_maps.append(m)
    return in_maps


def _prep_launch2(res1, w_o):
    # res1: list of 8 dicts with qat [1024,512], kat [256,512], vo [512,256]
    QT = np.zeros((B, LAT, S), BF)
    KT = np.zeros((B, KVD, S), BF)
    V = np.zeros((B, S, KVD), BF)
    for core in range(NCORES):
        bb, g = core // 4, core % 4
        r = res1[core]
        for ch, q0 in enumerate(_chunk_starts(g)):
            QT[bb, :, q0:q0 + CH] = r['qat'][:, CH * ch:CH * (ch + 1)]
            KT[bb, :, q0:q0 + CH] = r['kat'][:, CH * ch:CH * (ch + 1)]
            V[bb, q0:q0 + CH] = r['vo'][CH * ch:CH * (ch + 1)]

    mc4 = np.zeros((128, 4, 512), BF)
    for k in range(4):
        mc4[:, k, :] = (128 * k + np.arange(128)[:, None]
                        <= np.arange(512)[None, :])

    # wo blobs per kvh: [128, 32, 2, 128]
    wo_blobs = []
    for kvh in range(NKV):
        blk = w_o[:, KVD * kvh:KVD * (kvh + 1)].astype(BF)   # [4096, 256]
        wo_blobs.append(np.ascontiguousarray(
            blk.reshape(32, 128, 2, 128).transpose(3, 0, 2, 1)))
        # [p, oc, lt, c] = blk[128oc+c, 128lt+p]

    in_maps = []
    for core in range(NCORES):
        bb, kvh = core // 4, core % 4
        Vsh = np.zeros((S, HD + 1), BF)
        Vsrc = V[bb]
        base = HD * kvh
        if base + HD <= KVD // 2:
            Vsh[:, :HD] = Vsrc[:, base:base + HD]
        else:
            Vsh[1:, :HD] = Vsrc[:-1, base:base + HD]
        Vsh[:, HD] = 1.0
        va2 = np.ascontiguousarray(
            Vsh.reshape(16, 128, HD + 1).transpose(1, 0, 2))

        qt2 = np.zeros((128, 4, S), BF)
        for hl in range(4):
            h = 4 * kvh + hl
            qt2[0:64, hl, :] = QT[bb, HD * h:HD * (h + 1)]
            qt2[64:128, hl, :] = QT[bb, HD * h:HD * (h + 1)]
        kt2 = np.zeros((128, S), BF)
        kt2[0:64] = KT[bb, HD * kvh:HD * (kvh + 1), :]
        kt2[64:128] = kt2[0:64]
        in_maps.append(dict(qt2=qt2, kt2=kt2, va2=va2, wo2=wo_blobs[kvh],
                            mc4=mc4))
    return in_maps


# ---------------------------------------------------------------------------
# entry point
# ---------------------------------------------------------------------------

def _get_progs(with_qc2b, with_kc2b):
    key = (with_qc2b, with_kc2b)
    if key not in _PROGS:
        _PROGS[key] = (_build_launch1(with_qc2b, with_kc2b), _build_launch2())
    return _PROGS[key]


def _run(nc, in_maps, **kw):
    from concourse.bass_utils import run_bass_kernel_spmd
    return run_bass_kernel_spmd(nc, in_maps, list(range(NCORES)), **kw)


def kernel(x, w_qkv, w_o, qc1_w, qc1_b, qc2_w, qc2_b, kc1_w, kc1_b,
           kc2_w, kc2_b, key_temp, _profile=False):
    args = [np.asarray(a, F32) for a in
            (x, w_qkv, w_o, qc1_w, qc1_b, qc2_w, qc2_b, kc1_w, kc1_b,
             kc2_w, kc2_b, key_temp)]
    (x, w_qkv, w_o, qc1_w, qc1_b, qc2_w, qc2_b, kc1_w, kc1_b,
     kc2_w, kc2_b, key_temp) = args

    with_qc2b = bool(np.any(qc2_b))
    with_kc2b = bool(np.any(kc2_b))
    nc1, nc2 = _get_progs(with_qc2b, with_kc2b)

    maps1 = _prep_launch1(x, w_qkv, qc1_w, qc1_b, qc2_w, qc2_b,
                          kc1_w, kc1_b, kc2_w, kc2_b, key_temp)
    kw = dict(trace=True) if _profile else {}
    r1 = _run(nc1, maps1, **kw)
    kernel.exec_ns_1 = r1.exec_time_ns
    maps2 = _prep_launch2(r1.results, w_o)
    r2 = _run(nc2, maps2, **kw)
    kernel.exec_ns_2 = r2.exec_time_ns

    out = np.zeros((B, S, DIM), F32)
    for bb in range(B):
        acc = np.zeros((DIM, S), F32)
        for kvh in range(NKV):
            acc += r2.results[4 * bb + kvh]['po'].astype(F32)
        out[bb] = acc.T
    return out

